# revision 1
# baseline (speedup 1.0000x reference)
"""Cross-attention without softmax on 8 trn2 NeuronCores.

Reference computes out = (X Wq^T) (C Wk^T)^T (C Wv^T) * D^-0.5 per batch.
With no softmax the product reassociates:

    out_b = X_b @ A_b,   A_b = scale * Wq^T Wk (C_b^T C_b) Wv^T

which collapses the O(Sq*Skv*D) attention into two O(S*D^2) matmuls plus
a few 128x128 products. Sharding: batch (4) x query-half (2) -> 8 cores;
each core redundantly computes its batch's G = C^T C (no collectives).

I/O is pre-cast to bf16 on the host (halves HBM traffic, kills on-chip
casts); accumulation stays fp32 in PSUM. Row-tiles are formed from a
permuted grouping (partition p holds DRAM rows p*r+j) so every DMA runs
2KB-contiguous per partition. G's row-sum and X-row/out-row mapping are
invariant to that permutation.
"""

import os
import sys
import types

import numpy as np

_TRN_REPO = "/opt/trn_rl_repo"
if _TRN_REPO not in sys.path and not any("trn_rl_repo" in p for p in sys.path):
    sys.path.insert(0, _TRN_REPO)

import ml_dtypes  # noqa: E402

import concourse.bass as bass  # noqa: E402
import concourse.mybir as mybir  # noqa: E402
import concourse.tile as tile  # noqa: E402
from concourse import bacc  # noqa: E402
from concourse.bass_utils import run_bass_kernel_spmd  # noqa: E402
from concourse.masks import make_identity  # noqa: E402

B, SQ, SKV, D = 4, 4096, 4096, 128
N_CORES = 8
SQ_SHARD = SQ // (N_CORES // B)  # 2048
SCALE = float(D) ** -0.5
F32 = mybir.dt.float32
BF16 = mybir.dt.bfloat16

# "bf16": bf16 I/O + compute (fp32 PSUM accum). "f32": fp32 everywhere.
COMPUTE = os.environ.get("KERNEL_COMPUTE", "bf16")

_CACHE: dict = {}


def _install_axon_ntff_shim():
    try:
        import antenv.axon_hooks  # noqa: F401

        return
    except Exception:
        pass
    try:
        from trn_agent_boot.trn_boot import _ntff_profile_via_ctypes

        import antenv

        hook = _ntff_profile_via_ctypes("/opt/axon/libaxon_pjrt.so")
        mod = types.ModuleType("antenv.axon_hooks")
        mod._hook = hook
        mod.get_axon_ntff_profile_hook = lambda: mod._hook

        def _set(h):
            mod._hook = h

        mod.set_axon_ntff_profile_hook = _set
        antenv.axon_hooks = mod
        sys.modules["antenv.axon_hooks"] = mod
    except Exception:
        pass

    try:
        import concourse.bass_utils as bu

        bu.upload_artifacts = lambda tmpdir: f"file://{tmpdir}"
    except Exception:
        pass


def build_tile():
    """One SPMD graph, same on all 8 cores. Per-core inputs:
    x (2048,128), ctx (4096,128), wq/wk/wv (128,128); output out (2048,128).
    """
    cdt = BF16 if COMPUTE == "bf16" else F32

    nc = bacc.Bacc(None, target_bir_lowering=False, debug=False)
    x_ext = nc.declare_dram_parameter("x", [SQ_SHARD, D], cdt, isOutput=False)
    c_ext = nc.declare_dram_parameter("ctx", [SKV, D], cdt, isOutput=False)
    wq_ext = nc.declare_dram_parameter("wq", [D, D], cdt, isOutput=False)
    wk_ext = nc.declare_dram_parameter("wk", [D, D], cdt, isOutput=False)
    wv_ext = nc.declare_dram_parameter("wv", [D, D], cdt, isOutput=False)
    out_ext = nc.declare_dram_parameter("out", [SQ_SHARD, D], cdt, isOutput=True)

    R = 8  # rows per partition in the permuted grouping
    CTX_ROWS = 128 * R  # 1024 rows per ctx chunk
    n_ctx_chunks = SKV // CTX_ROWS  # 4
    n_x_chunks = SQ_SHARD // CTX_ROWS  # 2
    OG = 4  # out tiles per store group

    with tile.TileContext(nc) as tc:
        with (
            tc.tile_pool(name="const", bufs=1) as cpool,
            tc.tile_pool(name="ctxp", bufs=4) as ctxpool,
            tc.tile_pool(name="xp", bufs=2) as xpool,
            tc.tile_pool(name="outp", bufs=2) as opool,
            tc.tile_pool(name="psA", bufs=2, space="PSUM") as psA,
            tc.tile_pool(name="psX", bufs=2, space="PSUM") as psX,
            tc.tile_pool(name="psO", bufs=2, space="PSUM") as psO,
        ):
            ident = cpool.tile([128, 128], cdt)
            make_identity(nc, ident[:])

            wq = cpool.tile([D, D], cdt)
            wk = cpool.tile([D, D], cdt)
            wv = cpool.tile([D, D], cdt)

            # ---- G = C^T C ----
            # ctx chunk c: partition p holds rows c*1024 + p*8 .. +7
            # (2KB-contiguous per partition); slice [:, j, :] is a valid
            # 128-row tile of the row-sum.
            g_ps = psA.tile([D, D], F32, tag="chain")
            cc_chunks = []
            for c in range(n_ctx_chunks):
                cc = ctxpool.tile([128, R, D], cdt, tag="ctx")
                src = c_ext[c * CTX_ROWS : (c + 1) * CTX_ROWS, :].rearrange(
                    "(p r) d -> p r d", p=128
                )
                nc.sync.dma_start(cc[:], src)
                cc_chunks.append(cc)
            for c in range(n_ctx_chunks):
                cc = cc_chunks[c]
                for j in range(R):
                    nc.tensor.matmul(
                        g_ps[:],
                        cc[:, j, :],
                        cc[:, j, :],
                        start=(c == 0 and j == 0),
                        stop=(c == n_ctx_chunks - 1 and j == R - 1),
                    )
            gs = cpool.tile([D, D], cdt)
            nc.vector.tensor_copy(gs[:], g_ps[:])

            # ---- x loads (issued early; consumed by transposes) ----
            nc.sync.dma_start(wq[:], wq_ext[:])
            nc.sync.dma_start(wk[:], wk_ext[:])
            nc.sync.dma_start(wv[:], wv_ext[:])
            x_chunks = []
            for c in range(n_x_chunks):
                xc = xpool.tile([128, R, D], cdt, tag="x")
                src = x_ext[c * CTX_ROWS : (c + 1) * CTX_ROWS, :].rearrange(
                    "(p r) d -> p r d", p=128
                )
                nc.sync.dma_start(xc[:], src)
                x_chunks.append(xc)

            # ---- chain: U = Wq^T Wk; UT; WvT; P = G WvT; A = scale*U P ----
            u_ps = psA.tile([D, D], F32, tag="chain")
            nc.tensor.matmul(u_ps[:], wq[:], wk[:], start=True, stop=True)
            us = cpool.tile([D, D], cdt)
            nc.vector.tensor_copy(us[:], u_ps[:])

            ut_ps = psA.tile([D, D], cdt, tag="chain")
            nc.tensor.transpose(ut_ps[:], us[:], ident[:])
            ut = cpool.tile([D, D], cdt)
            nc.vector.tensor_copy(ut[:], ut_ps[:])

            wvt_ps = psA.tile([D, D], cdt, tag="chain")
            nc.tensor.transpose(wvt_ps[:], wv[:], ident[:])
            wvt = cpool.tile([D, D], cdt)
            nc.vector.tensor_copy(wvt[:], wvt_ps[:])

            p_ps = psA.tile([D, D], F32, tag="chain")
            nc.tensor.matmul(p_ps[:], gs[:], wvt[:], start=True, stop=True)
            ps = cpool.tile([D, D], cdt)
            nc.vector.tensor_copy(ps[:], p_ps[:])

            a_ps = psA.tile([D, D], F32, tag="chain")
            nc.tensor.matmul(a_ps[:], ut[:], ps[:], start=True, stop=True)
            a_sb = cpool.tile([D, D], cdt)
            nc.vector.tensor_copy(a_sb[:], a_ps[:])

            # ---- out rows: groups of OG tiles ----
            for c in range(n_x_chunks):
                xc = x_chunks[c]
                for g in range(R // OG):
                    xt_ps = psX.tile([D, OG * 128], cdt, tag="xtp")
                    for j in range(OG):
                        nc.tensor.transpose(
                            xt_ps[:, j * 128 : (j + 1) * 128],
                            xc[:, g * OG + j, :],
                            ident[:],
                        )
                    xt_sb = xpool.tile([D, OG * 128], cdt, tag="xt")
                    nc.vector.tensor_copy(xt_sb[:], xt_ps[:])

                    o_ps = psO.tile([128, OG * D], F32, tag="ops")
                    for j in range(OG):
                        nc.tensor.matmul(
                            o_ps[:, j * D : (j + 1) * D],
                            xt_sb[:, j * 128 : (j + 1) * 128],
                            a_sb[:],
                            start=True,
                            stop=True,
                        )
                    o_sb = opool.tile([128, OG, D], cdt, tag="osb")
                    nc.vector.tensor_copy(
                        o_sb[:].rearrange("p n d -> p (n d)"), o_ps[:]
                    )
                    dst = out_ext[
                        c * CTX_ROWS : (c + 1) * CTX_ROWS, :
                    ].rearrange("(p r) d -> p r d", p=128)[
                        :, g * OG : (g + 1) * OG, :
                    ]
                    nc.sync.dma_start(dst, o_sb[:])

    nc.compile()
    return nc


def build_raw():
    """Hand-scheduled raw-bass version: no Tile start/tail barriers.

    DMA issue is spread over four engines (sync: ctx0/1 + out stores,
    vector: ctx2/3, gpsimd: x0/1, scalar: wq/wk/ident/wv) because each
    HWDGE trigger costs ~600ns of sequencer time. The identity matrix is
    a host-provided input. PE order interleaves X-transpose groups into
    the slots where it would stall waiting for the next ctx chunk.

    Cumulative semaphore schedules (idx = value after the op):
      PE  (s_pe):  U1 UT2 WvT3 Gc0 4-11 Gc1 12-19 Tg1 20-23 Gc2 24-31
                   Tg2 32-35 Gc3 36-43 Tg3 44-47 P48 Tg4 49-52 A53
                   outg1 54-57 g2 58-61 g3 62-65 g4 66-69
      DVE (s_dve): us1 ut2 wvt3 xt1_4 xt2_5 gs6 ps7 xt3_8 a9 xt4_10
                   o1_11 o2_12 o3_13 o4_14

    PSUM banks: b0=G | b1=U,P,A | b2=UT,WvT | b3=xt1,xt2 | b4=xt3,xt4 |
    b5=o1,o4 | b6=o2 | b7=o3. Same-bank PE-write vs DVE-read pairs are
    serialized by the s_dve waits marked below (P10).
    """
    from contextlib import ExitStack

    cdt = BF16 if COMPUTE == "bf16" else F32
    assert cdt is BF16, "raw impl assumes bf16 I/O"

    nc = bacc.Bacc(None, target_bir_lowering=False, debug=False)
    x_ext = nc.declare_dram_parameter("x", [SQ_SHARD, D], cdt, isOutput=False)
    c_ext = nc.declare_dram_parameter("ctx", [SKV, D], cdt, isOutput=False)
    wq_ext = nc.declare_dram_parameter("wq", [D, D], cdt, isOutput=False)
    wk_ext = nc.declare_dram_parameter("wk", [D, D], cdt, isOutput=False)
    wvt_ext = nc.declare_dram_parameter("wvt", [D, D], cdt, isOutput=False)
    id_ext = nc.declare_dram_parameter("ident", [D, D], cdt, isOutput=False)
    out_ext = nc.declare_dram_parameter("out", [SQ_SHARD, D], cdt, isOutput=True)

    R = 8
    CTX_ROWS = 128 * R  # 1024
    RC = 8  # rows per partition per ctx chunk
    NCC = SKV // CTX_ROWS  # 4 ctx chunks
    NXC = SQ_SHARD // CTX_ROWS  # 2 x chunks

    ctx_view = [
        c_ext[c * CTX_ROWS : (c + 1) * CTX_ROWS, :].rearrange(
            "(p r) d -> p r d", p=128
        )
        for c in range(NCC)
    ]
    x_view = [
        x_ext[c * CTX_ROWS : (c + 1) * CTX_ROWS, :].rearrange(
            "(p r) d -> p r d", p=128
        )
        for c in range(NXC)
    ]
    out_view = [
        out_ext[c * CTX_ROWS : (c + 1) * CTX_ROWS, :].rearrange(
            "(p r) d -> p r d", p=128
        )
        for c in range(NXC)
    ]

    es = ExitStack()
    _n = [0]

    def sb(shape, dt, name=None):
        _n[0] += 1
        return es.enter_context(
            nc.sbuf_tensor(name or f"sb{_n[0]}", shape, dt)
        )

    def pst(shape, dt, name=None):
        _n[0] += 1
        return es.enter_context(
            nc.psum_tensor(name or f"ps{_n[0]}", shape, dt)
        )

    def sem(name):
        return es.enter_context(nc.semaphore(name))

    with es:
        ident = sb([128, 128], cdt, "ident_sb")
        wq = sb([D, D], cdt, "wq_sb")
        wk = sb([D, D], cdt, "wk_sb")
        wvt = sb([D, D], cdt, "wvt_sb")
        cc = [sb([128, RC, D], cdt, f"cc{i}") for i in range(NCC)]
        xch = [sb([128, R, D], cdt, f"xch{i}") for i in range(NXC)]
        gs = sb([D, D], cdt, "gs")
        ut = sb([D, D], cdt, "ut")
        pss = sb([D, D], cdt, "pss")
        a_sb = sb([D, D], cdt, "a_sb")
        xt_sb = [sb([D, 512], cdt, f"xt_sb{i}") for i in range(4)]
        o_sb = [sb([128, 4, D], cdt, f"o_sb{i}") for i in range(4)]

        g_ps = pst([128, 512], F32)  # b0 (use [:, :128])
        upa_ps = pst([128, 512], F32)  # b1: UT [:, :128], P [:,128:256], A [:,256:384]
        o4_ps = pst([128, 512], F32)  # b2
        xt12_ps = pst([128, 1024], cdt)  # b3
        xt34_ps = pst([128, 1024], cdt)  # b4
        o1_ps = pst([128, 512], F32)  # b5
        o2_ps = pst([128, 512], F32)  # b6
        o3_ps = pst([128, 512], F32)  # b7

        s_pe = sem("s_pe")
        s_dve = sem("s_dve")
        s_w = sem("s_w")
        s_c = [sem(f"s_c{i}") for i in range(NCC)]
        s_x = [sem(f"s_x{i}") for i in range(NXC)]
        s_st = sem("s_st")

        with nc.Block() as block:

            @block.sync
            def _(sync):
                nc.sync.dma_start(cc[0][:], ctx_view[0]).then_inc(s_c[0], 16)
                nc.sync.dma_start(cc[1][:], ctx_view[1]).then_inc(s_c[1], 16)
                for k in (0, 2, 4, 6):
                    nc.sync.wait_ge(s_dve, 9 + k)
                    dst = out_view[k // 4][:, (2 * k) % 8 : (2 * k) % 8 + 2, :]
                    src = o_sb[k // 2][:, (k % 2) * 2 : (k % 2) * 2 + 2, :]
                    nc.sync.dma_start(dst, src).then_inc(s_st, 16)
                nc.sync.wait_ge(s_st, 128)

            @block.scalar
            def _(sc):
                nc.scalar.dma_start(cc[2][:], ctx_view[2]).then_inc(s_c[2], 16)
                nc.scalar.dma_start(cc[3][:], ctx_view[3]).then_inc(s_c[3], 16)
                for k in (1, 3, 5, 7):
                    nc.scalar.wait_ge(s_dve, 9 + k)
                    dst = out_view[k // 4][:, (2 * k) % 8 : (2 * k) % 8 + 2, :]
                    src = o_sb[k // 2][:, (k % 2) * 2 : (k % 2) * 2 + 2, :]
                    nc.scalar.dma_start(dst, src).then_inc(s_st, 16)
                nc.scalar.wait_ge(s_st, 128)

            @block.gpsimd
            def _(gp):
                nc.gpsimd.dma_start(wq[:], wq_ext[:]).then_inc(s_w, 16)
                nc.gpsimd.dma_start(wk[:], wk_ext[:]).then_inc(s_w, 16)
                nc.gpsimd.dma_start(ident[:], id_ext[:]).then_inc(s_w, 16)
                nc.gpsimd.dma_start(wvt[:], wvt_ext[:]).then_inc(s_w, 16)
                nc.gpsimd.dma_start(xch[0][:], x_view[0]).then_inc(s_x[0], 16)
                nc.gpsimd.dma_start(xch[1][:], x_view[1]).then_inc(s_x[1], 16)

            @block.tensor
            def _(te):
                def gchunk(c):
                    nc.tensor.wait_ge(s_c[c], 16)
                    for j in range(RC):
                        nc.tensor.matmul(
                            g_ps[:, :128],
                            cc[c][:, j, :],
                            cc[c][:, j, :],
                            start=(c == 0 and j == 0),
                            stop=(c == NCC - 1 and j == RC - 1),
                        ).then_inc(s_pe, 1)

                def tgroup(ps_ap, xc, base):
                    for j in range(4):
                        nc.tensor.transpose(
                            ps_ap[:, j * 128 : (j + 1) * 128],
                            xc[:, base + j, :],
                            ident[:],
                        ).then_inc(s_pe, 1)

                gchunk(0)  # 1..8
                gchunk(1)  # 9..16
                gchunk(2)  # 17..24
                # 25: UT = Wk^T Wq' slotted before the last G chunk
                # (w landed long ago; ut copy overlaps Gc3 on the DVE)
                nc.tensor.wait_ge(s_w, 64)
                nc.tensor.matmul(
                    upa_ps[:, :128], wk[:], wq[:], start=True, stop=True
                ).then_inc(s_pe, 1)
                gchunk(3)  # 26..33
                nc.tensor.wait_ge(s_x[0], 16)
                tgroup(xt12_ps[:, :512], xch[0], 0)  # Tg1 34..37
                # 38: P = G WvT (s_dve>=2: ut+gs copied; b1 P10)
                nc.tensor.wait_ge(s_dve, 2)
                nc.tensor.matmul(
                    upa_ps[:, 128:256], gs[:], wvt[:], start=True, stop=True
                ).then_inc(s_pe, 1)
                nc.tensor.wait_ge(s_dve, 3)  # xt1 copied (b3, P10)
                tgroup(xt12_ps[:, 512:], xch[0], 4)  # Tg2 39..42
                # 43: A = UT P
                nc.tensor.wait_ge(s_dve, 4)  # ps copied
                nc.tensor.matmul(
                    upa_ps[:, 256:384], ut[:], pss[:], start=True, stop=True
                ).then_inc(s_pe, 1)
                nc.tensor.wait_ge(s_x[1], 16)
                tgroup(xt34_ps[:, :512], xch[1], 0)  # Tg3 44..47
                nc.tensor.wait_ge(s_dve, 7)  # xt3 copied (b4, P10)
                tgroup(xt34_ps[:, 512:], xch[1], 4)  # Tg4 48..51
                # 52..67: out in 8 two-tile groups, banks cycle
                # [b5,b6,b7,b2]; second pass waits that bank's copy (P10).
                o_banks = [o1_ps, o2_ps, o3_ps, o4_ps]
                for k in range(8):
                    if k == 0:
                        nc.tensor.wait_ge(s_dve, 5)  # a_sb copied
                    if k == 2:
                        nc.tensor.wait_ge(s_dve, 6)  # xt2 copied
                    if k >= 4:
                        nc.tensor.wait_ge(s_dve, 9 + (k - 4))  # bank freed
                    bank = o_banks[k % 4]
                    half = (k // 4) * 256
                    xt = xt_sb[k // 2]
                    xoff = (k % 2) * 256
                    for j in range(2):
                        nc.tensor.matmul(
                            bank[:, half + j * D : half + (j + 1) * D],
                            xt[:, xoff + j * 128 : xoff + (j + 1) * 128],
                            a_sb[:],
                            start=True,
                            stop=True,
                        ).then_inc(s_pe, 1)

            @block.vector
            def _(ve):
                def vcopy(dst, src, pe_thresh):
                    nc.vector.wait_ge(s_pe, pe_thresh)
                    nc.vector.tensor_copy(dst, src).then_inc(s_dve, 1)

                vcopy(ut[:], upa_ps[:, :128], 25)  # 1
                vcopy(gs[:], g_ps[:, :128], 33)  # 2
                vcopy(xt_sb[0][:], xt12_ps[:, :512], 37)  # 3
                vcopy(pss[:], upa_ps[:, 128:256], 38)  # 4
                vcopy(a_sb[:], upa_ps[:, 256:384], 43)  # 5
                vcopy(xt_sb[1][:], xt12_ps[:, 512:], 42)  # 6
                vcopy(xt_sb[2][:], xt34_ps[:, :512], 47)  # 7
                vcopy(xt_sb[3][:], xt34_ps[:, 512:], 51)  # 8
                o_banks = [o1_ps, o2_ps, o3_ps, o4_ps]
                for k in range(8):  # 9..16
                    bank = o_banks[k % 4]
                    half = (k // 4) * 256
                    vcopy(
                        o_sb[k // 2][
                            :, (k % 2) * 2 : (k % 2) * 2 + 2, :
                        ].rearrange("p n d -> p (n d)"),
                        bank[:, half : half + 256],
                        53 + 2 * k,
                    )

    nc.compile()
    return nc


def build():
    if os.environ.get("KERNEL_IMPL", "raw") == "raw":
        return build_raw()
    return build_tile()


def _get_nc():
    if "nc" not in _CACHE:
        _CACHE["nc"] = build()
    return _CACHE["nc"]


def _run(inputs: dict, trace: bool = False, **kw):
    np_dt = ml_dtypes.bfloat16 if COMPUTE == "bf16" else np.float32
    context = np.ascontiguousarray(inputs["context"]).astype(np_dt)
    X = np.ascontiguousarray(inputs["X"]).astype(np_dt)
    Wq = (np.ascontiguousarray(inputs["Wq"]).astype(np.float32) * SCALE).astype(np_dt)
    Wk = np.ascontiguousarray(inputs["Wk"]).astype(np_dt)
    Wv = np.ascontiguousarray(inputs["Wv"]).astype(np_dt)
    Wvt = np.ascontiguousarray(np.asarray(inputs["Wv"]).T).astype(np_dt)

    raw = os.environ.get("KERNEL_IMPL", "raw") == "raw"
    ident = np.eye(D, dtype=np_dt)
    in_maps = []
    for c in range(N_CORES):
        b, h = divmod(c, 2)
        m = {
            "x": np.ascontiguousarray(
                X[b, h * SQ_SHARD : (h + 1) * SQ_SHARD, :]
            ),
            "ctx": np.ascontiguousarray(context[b]),
            "wq": Wq,
            "wk": Wk,
        }
        if raw:
            m["wvt"] = Wvt
            m["ident"] = ident
        else:
            m["wv"] = Wv
        in_maps.append(m)

    nc = _get_nc()
    res = run_bass_kernel_spmd(
        nc, in_maps, core_ids=list(range(N_CORES)), trace=trace, **kw
    )
    out = np.empty((B, SQ, D), dtype=np.float32)
    for c in range(N_CORES):
        b, h = divmod(c, 2)
        out[b, h * SQ_SHARD : (h + 1) * SQ_SHARD, :] = res.results[c][
            "out"
        ].astype(np.float32)
    return out, res


def kernel(**inputs: np.ndarray) -> np.ndarray:
    if os.environ.get("BASS_TRACE"):
        _install_axon_ntff_shim()
    try:
        out, _ = _run(inputs, trace=False)
    except Exception:
        # transient NRT device errors have been observed once across many
        # runs; one retry on a fresh execution
        out, _ = _run(inputs, trace=False)
    return out


if __name__ == "__main__":
    rng = np.random.default_rng(0)
    ins = {
        "context": rng.standard_normal((B, SKV, D)).astype(np.float32),
        "X": rng.standard_normal((B, SQ, D)).astype(np.float32),
        "Wq": (rng.standard_normal((D, D)) / np.sqrt(D)).astype(np.float32),
        "Wk": (rng.standard_normal((D, D)) / np.sqrt(D)).astype(np.float32),
        "Wv": (rng.standard_normal((D, D)) / np.sqrt(D)).astype(np.float32),
    }
    got = kernel(**ins)
    q = ins["X"] @ ins["Wq"].T
    k = ins["context"] @ ins["Wk"].T
    v = ins["context"] @ ins["Wv"].T
    w = np.einsum("bse,bte->bst", q, k) * SCALE
    want = np.einsum("bst,bte->bse", w, v)
    rel = np.linalg.norm(got - want) / np.linalg.norm(want)
    print("rel err vs numpy:", rel)



# revision 2
# speedup vs baseline: 1.0349x; 1.0349x over previous
"""Cross-attention without softmax on 8 trn2 NeuronCores.

Reference computes out = (X Wq^T) (C Wk^T)^T (C Wv^T) * D^-0.5 per batch.
With no softmax the product reassociates:

    out_b = X_b @ U @ G_b @ W2,  U = Wq^T Wk,  G_b = C_b^T C_b,
                                 W2 = D^-0.5 Wv^T

U and W2 are weight-only and precomputed on the host. The device computes
G (32 accumulating 128x128 matmuls fed by the ctx DMA stream),
Q'^T = U^T X^T (four 512-wide matmuls against a host-pre-transposed X,
run in the DMA shadow), V = G W2 (one matmul; G is symmetric so it is its
own lhsT), and out = Q' V (16 matmuls feeding 4 overlapping stores).
No on-device transposes, no identity matrix, minimal PSUM round-trips.

Sharding: batch (4) x query-half (2) -> 8 cores; each core redundantly
computes its batch's G (no collectives). I/O is pre-cast to bf16 on the
host; accumulation stays fp32 in PSUM.

Layouts: ctx row-tiles use the permuted grouping (partition p holds DRAM
rows c*1024 + p*8 + j) so ctx DMA runs 2KB-contiguous per partition; G's
row-sum is invariant to that permutation. Out rows are stored in a
device-friendly permuted order (dev row g*512+p*4+j holds true row
g*512+j*128+p, 1KB-contiguous stores) and un-permuted on the host.
"""

import os
import sys
import types

import numpy as np

_TRN_REPO = "/opt/trn_rl_repo"
if _TRN_REPO not in sys.path and not any("trn_rl_repo" in p for p in sys.path):
    sys.path.insert(0, _TRN_REPO)

import ml_dtypes  # noqa: E402

import concourse.bass as bass  # noqa: E402
import concourse.mybir as mybir  # noqa: E402
from concourse import bacc  # noqa: E402
from concourse.bass_utils import run_bass_kernel_spmd  # noqa: E402

B, SQ, SKV, D = 4, 4096, 4096, 128
N_CORES = 8
SQ_SHARD = SQ // (N_CORES // B)  # 2048
SCALE = float(D) ** -0.5
F32 = mybir.dt.float32
BF16 = mybir.dt.bfloat16

_CACHE: dict = {}


def _install_axon_ntff_shim():
    try:
        import antenv.axon_hooks  # noqa: F401

        return
    except Exception:
        pass
    try:
        from trn_agent_boot.trn_boot import _ntff_profile_via_ctypes

        import antenv

        hook = _ntff_profile_via_ctypes("/opt/axon/libaxon_pjrt.so")
        mod = types.ModuleType("antenv.axon_hooks")
        mod._hook = hook
        mod.get_axon_ntff_profile_hook = lambda: mod._hook

        def _set(h):
            mod._hook = h

        mod.set_axon_ntff_profile_hook = _set
        antenv.axon_hooks = mod
        sys.modules["antenv.axon_hooks"] = mod
    except Exception:
        pass

    try:
        import concourse.bass_utils as bu

        bu.upload_artifacts = lambda tmpdir: f"file://{tmpdir}"
    except Exception:
        pass


def build_raw():
    """Hand-scheduled raw-bass kernel. Per-core inputs:
    xt (128, 2048) = X-shard transposed, ctx (4096, 128),
    w (128, 256) = [U | W2]; output out (2048, 128) in permuted row order.

    DMA: sync HW queue: ctx chunks 0,1 + stores o2,o3; scalar HW queue:
    ctx chunks 2,3 + stores o0,o1 (plus o0/o1 PSUM copies); gpsimd SW
    queue: w, xt halves. PE processes ctx chunks in arrival order
    (0,2,1,3) and slots the Q'^T matmuls into the DMA shadow.

    Cumulative semaphore schedules (value after the op):
      PE (s_pe): Gc0 1-8, Gc2 9-16, QT0 17, QT1 18, Gc1 19-26, QT2 27,
                 QT3 28, Gc3 29-36, V 37, out 38-53 (4 groups of 4)
      DVE (s_dve): qt0 1, qt1 2, qt2 3, qt3 4, gs 5, v 6, o2 7, o3 8

    PSUM banks: b0 = G [:, :128] + V [:, 128:256]; b1-b4 = QT0-3
    (b1 reused for out group 3); b5-b7 = out groups 0-2. Same-bank
    PE-write vs engine-read pairs are serialized by the waits marked.
    """
    from contextlib import ExitStack

    cdt = BF16

    nc = bacc.Bacc(None, target_bir_lowering=False, debug=False)
    xt_ext = nc.declare_dram_parameter("xt", [D, SQ_SHARD], cdt, isOutput=False)
    c_ext = nc.declare_dram_parameter("ctx", [SKV, D], cdt, isOutput=False)
    w_ext = nc.declare_dram_parameter("w", [D, 2 * D], cdt, isOutput=False)
    out_ext = nc.declare_dram_parameter("out", [SQ_SHARD, D], cdt, isOutput=True)

    R = 8
    CTX_ROWS = 128 * R  # 1024
    NCC = SKV // CTX_ROWS  # 4 ctx chunks
    NOG = 4  # out store groups
    OT = SQ_SHARD // 128 // NOG  # 4 tiles per group

    ctx_view = [
        c_ext[c * CTX_ROWS : (c + 1) * CTX_ROWS, :].rearrange(
            "(p r) d -> p r d", p=128
        )
        for c in range(NCC)
    ]
    out_view = [
        out_ext[g * 512 : (g + 1) * 512, :].rearrange("(p r) d -> p r d", p=128)
        for g in range(NOG)
    ]

    es = ExitStack()
    _n = [0]

    def sb(shape, dt, name=None):
        _n[0] += 1
        return es.enter_context(nc.sbuf_tensor(name or f"sb{_n[0]}", shape, dt))

    def pst(shape, dt, name=None):
        _n[0] += 1
        return es.enter_context(nc.psum_tensor(name or f"ps{_n[0]}", shape, dt))

    def sem(name):
        return es.enter_context(nc.semaphore(name))

    with es:
        w_sb = sb([D, 2 * D], cdt, "w_sb")
        cc = [sb([128, R, D], cdt, f"cc{i}") for i in range(NCC)]
        xt_sb = sb([D, SQ_SHARD], cdt, "xt_sb")
        gs = sb([D, D], cdt, "gs")
        v_sb = sb([D, D], cdt, "v_sb")
        qts = sb([D, SQ_SHARD], cdt, "qts")
        o_sb = [sb([128, OT, D], cdt, f"o_sb{i}") for i in range(NOG)]

        gv_ps = pst([128, 512], F32)  # b0: G [:, :128], V [:, 128:256]
        qt_ps = [pst([128, 512], F32) for _ in range(4)]  # b1-b4
        o_ps = [pst([128, 512], F32) for _ in range(3)]  # b5-b7
        o_banks = o_ps + [qt_ps[0]]  # out group 3 reuses b1

        s_pe = sem("s_pe")
        s_dve = sem("s_dve")
        s_w = sem("s_w")
        s_c = [sem(f"s_c{i}") for i in range(NCC)]
        s_x = [sem(f"s_x{i}") for i in range(2)]
        s_st = sem("s_st")

        U = w_sb[:, :D]
        W2 = w_sb[:, D:]
        # PE processes ctx chunks in DMA arrival order
        PE_CTX_ORDER = (0, 2, 1, 3)

        with nc.Block() as block:

            @block.sync
            def _(sync):
                nc.sync.dma_start(cc[0][:], ctx_view[0]).then_inc(s_c[0], 16)
                nc.sync.dma_start(cc[1][:], ctx_view[1]).then_inc(s_c[1], 16)
                nc.sync.wait_ge(s_dve, 7)  # o2 copied
                nc.sync.dma_start(out_view[2], o_sb[2][:]).then_inc(s_st, 16)
                nc.sync.wait_ge(s_dve, 8)  # o3 copied
                nc.sync.dma_start(out_view[3], o_sb[3][:]).then_inc(s_st, 16)
                nc.sync.wait_ge(s_st, 64)

            @block.scalar
            def _(sc):
                nc.scalar.dma_start(cc[2][:], ctx_view[2]).then_inc(s_c[2], 16)
                nc.scalar.dma_start(cc[3][:], ctx_view[3]).then_inc(s_c[3], 16)
                nc.scalar.wait_ge(s_pe, 41)  # out group 0 done (b5)
                nc.scalar.copy(
                    o_sb[0][:].rearrange("p n d -> p (n d)"), o_ps[0][:]
                )
                nc.scalar.dma_start(out_view[0], o_sb[0][:]).then_inc(s_st, 16)
                nc.scalar.wait_ge(s_pe, 45)  # out group 1 done (b6)
                nc.scalar.copy(
                    o_sb[1][:].rearrange("p n d -> p (n d)"), o_ps[1][:]
                )
                nc.scalar.dma_start(out_view[1], o_sb[1][:]).then_inc(s_st, 16)
                nc.scalar.wait_ge(s_st, 64)

            @block.gpsimd
            def _(gp):
                nc.gpsimd.dma_start(w_sb[:], w_ext[:]).then_inc(s_w, 16)
                nc.gpsimd.dma_start(
                    xt_sb[:, : SQ_SHARD // 2], xt_ext[:, : SQ_SHARD // 2]
                ).then_inc(s_x[0], 16)
                nc.gpsimd.dma_start(
                    xt_sb[:, SQ_SHARD // 2 :], xt_ext[:, SQ_SHARD // 2 :]
                ).then_inc(s_x[1], 16)

            @block.tensor
            def _(te):
                def gchunk(c, first, last):
                    nc.tensor.wait_ge(s_c[c], 16)
                    for j in range(R):
                        nc.tensor.matmul(
                            gv_ps[:, :128],
                            cc[c][:, j, :],
                            cc[c][:, j, :],
                            start=(first and j == 0),
                            stop=(last and j == R - 1),
                        ).then_inc(s_pe, 1)

                def qt(c):
                    nc.tensor.matmul(
                        qt_ps[c][:],
                        U,
                        xt_sb[:, c * 512 : (c + 1) * 512],
                        start=True,
                        stop=True,
                    ).then_inc(s_pe, 1)

                gchunk(PE_CTX_ORDER[0], True, False)  # 1-8
                gchunk(PE_CTX_ORDER[1], False, False)  # 9-16
                nc.tensor.wait_ge(s_w, 16)
                nc.tensor.wait_ge(s_x[0], 16)
                qt(0)  # 17
                qt(1)  # 18
                gchunk(PE_CTX_ORDER[2], False, False)  # 19-26
                nc.tensor.wait_ge(s_x[1], 16)
                qt(2)  # 27
                qt(3)  # 28
                gchunk(PE_CTX_ORDER[3], False, True)  # 29-36
                # V = G W2 (G symmetric -> its bf16 copy is its own lhsT).
                # s_dve>=5: gs copied, also serializes b0 PE-write vs DVE-read
                nc.tensor.wait_ge(s_dve, 5)
                nc.tensor.matmul(
                    gv_ps[:, 128:256], gs[:], W2, start=True, stop=True
                ).then_inc(s_pe, 1)  # 37
                nc.tensor.wait_ge(s_dve, 6)  # v copied
                for k in range(16):  # 38-53
                    bank = o_banks[k // 4]
                    nc.tensor.matmul(
                        bank[:, (k % 4) * D : (k % 4 + 1) * D],
                        qts[:, k * 128 : (k + 1) * 128],
                        v_sb[:],
                        start=True,
                        stop=True,
                    ).then_inc(s_pe, 1)

            @block.vector
            def _(ve):
                def vcopy(dst, src, pe_thresh):
                    nc.vector.wait_ge(s_pe, pe_thresh)
                    nc.vector.tensor_copy(dst, src).then_inc(s_dve, 1)

                vcopy(qts[:, :512], qt_ps[0][:], 17)  # 1
                vcopy(qts[:, 512:1024], qt_ps[1][:], 18)  # 2
                vcopy(qts[:, 1024:1536], qt_ps[2][:], 27)  # 3
                vcopy(qts[:, 1536:], qt_ps[3][:], 28)  # 4
                vcopy(gs[:], gv_ps[:, :128], 36)  # 5
                vcopy(v_sb[:], gv_ps[:, 128:256], 37)  # 6
                vcopy(
                    o_sb[2][:].rearrange("p n d -> p (n d)"), o_ps[2][:], 49
                )  # 7
                vcopy(
                    o_sb[3][:].rearrange("p n d -> p (n d)"), qt_ps[0][:], 53
                )  # 8

    nc.compile()
    return nc


def _get_nc():
    if "nc" not in _CACHE:
        _CACHE["nc"] = build_raw()
    return _CACHE["nc"]


def _prep_in_maps(inputs: dict):
    bf16 = ml_dtypes.bfloat16
    context = np.ascontiguousarray(inputs["context"]).astype(bf16)
    X = np.ascontiguousarray(inputs["X"]).astype(np.float32)
    Wq = np.ascontiguousarray(inputs["Wq"]).astype(np.float32)
    Wk = np.ascontiguousarray(inputs["Wk"]).astype(np.float32)
    Wv = np.ascontiguousarray(inputs["Wv"]).astype(np.float32)

    U = Wq.T @ Wk
    W2 = SCALE * Wv.T
    w_host = np.ascontiguousarray(
        np.concatenate([U, W2], axis=1).astype(bf16)
    )

    Xb = X.astype(bf16)
    in_maps = []
    for c in range(N_CORES):
        b, h = divmod(c, 2)
        xt = np.ascontiguousarray(
            Xb[b, h * SQ_SHARD : (h + 1) * SQ_SHARD, :].T
        )
        in_maps.append(
            {"xt": xt, "ctx": np.ascontiguousarray(context[b]), "w": w_host}
        )
    return in_maps


def _unpermute(dev: np.ndarray) -> np.ndarray:
    # dev row g*512 + p*4 + j holds true row g*512 + j*128 + p
    return (
        dev.reshape(4, 128, 4, D).transpose(0, 2, 1, 3).reshape(SQ_SHARD, D)
    )


def _run(inputs: dict, trace: bool = False, **kw):
    in_maps = _prep_in_maps(inputs)
    nc = _get_nc()
    res = run_bass_kernel_spmd(
        nc, in_maps, core_ids=list(range(N_CORES)), trace=trace, **kw
    )
    out = np.empty((B, SQ, D), dtype=np.float32)
    for c in range(N_CORES):
        b, h = divmod(c, 2)
        out[b, h * SQ_SHARD : (h + 1) * SQ_SHARD, :] = _unpermute(
            res.results[c]["out"]
        ).astype(np.float32)
    return out, res


def kernel(**inputs: np.ndarray) -> np.ndarray:
    if os.environ.get("BASS_TRACE"):
        _install_axon_ntff_shim()
    try:
        out, _ = _run(inputs, trace=False)
    except Exception:
        # transient NRT device errors have been observed once across many
        # runs; one retry on a fresh execution
        out, _ = _run(inputs, trace=False)
    return out


if __name__ == "__main__":
    rng = np.random.default_rng(0)
    ins = {
        "context": rng.standard_normal((B, SKV, D)).astype(np.float32),
        "X": rng.standard_normal((B, SQ, D)).astype(np.float32),
        "Wq": (rng.standard_normal((D, D)) / np.sqrt(D)).astype(np.float32),
        "Wk": (rng.standard_normal((D, D)) / np.sqrt(D)).astype(np.float32),
        "Wv": (rng.standard_normal((D, D)) / np.sqrt(D)).astype(np.float32),
    }
    got = kernel(**ins)
    q = ins["X"] @ ins["Wq"].T
    k = ins["context"] @ ins["Wk"].T
    v = ins["context"] @ ins["Wv"].T
    w = np.einsum("bse,bte->bst", q, k) * SCALE
    want = np.einsum("bst,bte->bse", w, v)
    rel = np.linalg.norm(got - want) / np.linalg.norm(want)
    print("rel err vs numpy:", rel)


# revision 7
# speedup vs baseline: 1.1251x; 1.0872x over previous
"""Cross-attention without softmax on 8 trn2 NeuronCores.

Reference computes out = (X Wq^T) (C Wk^T)^T (C Wv^T) * D^-0.5 per batch.
With no softmax the product reassociates:

    out_b = X_b @ P2_b,  P2_b = U G_b W2,  U = Wq^T Wk,
    G_b = C_b^T C_b,     W2 = D^-0.5 Wv^T

U and W2 are weight-only and precomputed on the host. The device computes
G (32 accumulating 128x128 matmuls alternating between two PSUM banks so
they pipeline, fed by the ctx DMA stream), then the tiny chain
V = G W2 (G is symmetric so its SBUF copy is its own lhsT) and
P2 = U V, and finally out = X P2 as 16 matmuls whose lhsT slices come
straight from a host-pre-transposed X — no on-device transposes, no
Q' intermediate, minimal PSUM round-trips.

Sharding: batch (4) x query-half (2) -> 8 cores; each core redundantly
computes its batch's G (no collectives). I/O is pre-cast to bf16 on the
host; accumulation stays fp32 in PSUM.

Layouts: ctx row-tiles use the permuted grouping (partition p holds rows
base + p*r + j) so ctx DMA runs >=512B-contiguous per partition; G's
row-sum is invariant to that permutation. Chunks shrink toward the end
(1024,1024,768,768,256,256 rows) so G finishes almost immediately after
the last ctx byte. Out rows are stored in a device-friendly permuted
order (dev row g*512+p*4+j holds true row g*512+j*128+p,
1KB-contiguous stores) and un-permuted on the host.
"""

import os
import sys
import types

import numpy as np

_TRN_REPO = "/opt/trn_rl_repo"
if _TRN_REPO not in sys.path and not any("trn_rl_repo" in p for p in sys.path):
    sys.path.insert(0, _TRN_REPO)

import ml_dtypes  # noqa: E402

import concourse.bass as bass  # noqa: E402
import concourse.mybir as mybir  # noqa: E402
from concourse import bacc  # noqa: E402
from concourse.bass_utils import run_bass_kernel_spmd  # noqa: E402

B, SQ, SKV, D = 4, 4096, 4096, 128
N_CORES = 8
SQ_SHARD = SQ // (N_CORES // B)  # 2048
SCALE = float(D) ** -0.5
F32 = mybir.dt.float32
BF16 = mybir.dt.bfloat16

_CACHE: dict = {}


def _install_axon_ntff_shim():
    try:
        import antenv.axon_hooks  # noqa: F401

        return
    except Exception:
        pass
    try:
        from trn_agent_boot.trn_boot import _ntff_profile_via_ctypes

        import antenv

        hook = _ntff_profile_via_ctypes("/opt/axon/libaxon_pjrt.so")
        mod = types.ModuleType("antenv.axon_hooks")
        mod._hook = hook
        mod.get_axon_ntff_profile_hook = lambda: mod._hook

        def _set(h):
            mod._hook = h

        mod.set_axon_ntff_profile_hook = _set
        antenv.axon_hooks = mod
        sys.modules["antenv.axon_hooks"] = mod
    except Exception:
        pass

    try:
        import concourse.bass_utils as bu

        bu.upload_artifacts = lambda tmpdir: f"file://{tmpdir}"
    except Exception:
        pass


# ctx chunk row counts; 128*r rows each, sum = 4096
CTX_R = (8, 8, 6, 6, 2, 2)
NCC = len(CTX_R)


def build_raw():
    """Hand-scheduled raw-bass kernel. Per-core inputs:
    xt (128, 2048) = X-shard transposed, ctx (4096, 128),
    w (128, 256) = [U^T | W2]; output out (2048, 128), permuted rows.

    DMA: sync HW queue: ctx chunks 0,2,4 + xt half 0, stores o0,o2;
    scalar HW queue: ctx chunks 1,3,5 + xt half 1, stores o1,o3 (plus
    o1/o3 PSUM copies); gpsimd SW queue: w only.

    Cumulative semaphore schedules (value after the op):
      PE (s_pe): G 1-32 (chunk ends at 8,16,22,28,30,32), V 33, P2 34,
                 out 35-50 (4 groups of 4, banks b4-b7)
      DVE (s_dve): gs merge 1, v 2, p2 3, o0 copy 4, o2 copy 5

    PSUM banks: b0/b1 = G even/odd accumulators; b2 = V; b3 = P2;
    b4-b7 = out groups 0-3. Same-bank PE-write vs engine-read pairs are
    serialized by the s_pe waits listed above.
    """
    from contextlib import ExitStack

    cdt = BF16

    nc = bacc.Bacc(None, target_bir_lowering=False, debug=False)
    xt_ext = nc.declare_dram_parameter("xt", [D, SQ_SHARD], cdt, isOutput=False)
    c_ext = nc.declare_dram_parameter("ctx", [SKV, D], cdt, isOutput=False)
    w_ext = nc.declare_dram_parameter("w", [D, 2 * D], cdt, isOutput=False)
    out_ext = nc.declare_dram_parameter("out", [SQ_SHARD, D], cdt, isOutput=True)

    NOG = 4  # out store groups
    OT = SQ_SHARD // 128 // NOG  # 4 tiles per group

    ctx_view = []
    row = 0
    for r in CTX_R:
        ctx_view.append(
            c_ext[row : row + 128 * r, :].rearrange("(p r) d -> p r d", p=128)
        )
        row += 128 * r
    out_view = [
        out_ext[g * 512 : (g + 1) * 512, :].rearrange("(p r) d -> p r d", p=128)
        for g in range(NOG)
    ]

    es = ExitStack()
    _n = [0]

    def sb(shape, dt, name=None):
        _n[0] += 1
        return es.enter_context(nc.sbuf_tensor(name or f"sb{_n[0]}", shape, dt))

    def pst(shape, dt, name=None):
        _n[0] += 1
        return es.enter_context(nc.psum_tensor(name or f"ps{_n[0]}", shape, dt))

    def sem(name):
        return es.enter_context(nc.semaphore(name))

    with es:
        w_sb = sb([D, 2 * D], cdt, "w_sb")
        cc = [sb([128, r, D], cdt, f"cc{i}") for i, r in enumerate(CTX_R)]
        xt_sb = sb([D, SQ_SHARD], cdt, "xt_sb")
        ga_sb = sb([D, D], F32, "ga_sb")
        gs = sb([D, D], cdt, "gs")
        v_sb = sb([D, D], cdt, "v_sb")
        p2_sb = sb([D, D], cdt, "p2_sb")
        o_sb = [sb([128, OT, D], cdt, f"o_sb{i}") for i in range(NOG)]

        ga_ps = pst([128, 512], F32)  # b0
        gb_ps = pst([128, 512], F32)  # b1
        v_ps = pst([128, 512], F32)  # b2
        p2_ps = pst([128, 512], F32)  # b3
        o_ps = [pst([128, 512], F32) for _ in range(NOG)]  # b4-b7

        s_pe = sem("s_pe")
        s_dve = sem("s_dve")
        s_w = sem("s_w")
        s_c = [sem(f"s_c{i}") for i in range(NCC)]
        s_x = [sem(f"s_x{i}") for i in range(2)]
        s_st = sem("s_st")

        UT = w_sb[:, :D]
        W2 = w_sb[:, D:]

        with nc.Block() as block:

            @block.sync
            def _(sync):
                for i in (0, 2, 4):
                    nc.sync.dma_start(cc[i][:], ctx_view[i]).then_inc(s_c[i], 16)
                nc.sync.dma_start(
                    xt_sb[:, : SQ_SHARD // 2], xt_ext[:, : SQ_SHARD // 2]
                ).then_inc(s_x[0], 16)
                nc.sync.wait_ge(s_dve, 5)  # o0 copied
                nc.sync.dma_start(out_view[0], o_sb[0][:]).then_inc(s_st, 16)
                nc.sync.wait_ge(s_dve, 6)  # o2 copied
                nc.sync.dma_start(out_view[2], o_sb[2][:]).then_inc(s_st, 16)
                nc.sync.wait_ge(s_st, 64)

            @block.scalar
            def _(sc):
                for i in (1, 3, 5):
                    nc.scalar.dma_start(cc[i][:], ctx_view[i]).then_inc(
                        s_c[i], 16
                    )
                nc.scalar.dma_start(
                    xt_sb[:, SQ_SHARD // 2 :], xt_ext[:, SQ_SHARD // 2 :]
                ).then_inc(s_x[1], 16)
                nc.scalar.wait_ge(s_pe, 42)  # out group 1 done (b5)
                nc.scalar.copy(
                    o_sb[1][:].rearrange("p n d -> p (n d)"), o_ps[1][:]
                )
                nc.scalar.dma_start(out_view[1], o_sb[1][:]).then_inc(s_st, 16)
                nc.scalar.wait_ge(s_pe, 50)  # out group 3 done (b7)
                nc.scalar.copy(
                    o_sb[3][:].rearrange("p n d -> p (n d)"), o_ps[3][:]
                )
                nc.scalar.dma_start(out_view[3], o_sb[3][:]).then_inc(s_st, 16)
                nc.scalar.wait_ge(s_st, 64)

            @block.gpsimd
            def _(gp):
                nc.gpsimd.dma_start(w_sb[:], w_ext[:]).then_inc(s_w, 16)

            @block.tensor
            def _(te):
                # G: alternate PSUM banks per matmul so consecutive matmuls
                # pipeline instead of serializing on one bank's accumulator
                m = 0
                n_mm = sum(CTX_R)
                for i, r in enumerate(CTX_R):
                    nc.tensor.wait_ge(s_c[i], 16)
                    for j in range(r):
                        bank = ga_ps if m % 2 == 0 else gb_ps
                        nc.tensor.matmul(
                            bank[:, :128],
                            cc[i][:, j, :],
                            cc[i][:, j, :],
                            start=(m < 2),
                            stop=(m >= n_mm - 2),
                        ).then_inc(s_pe, 1)
                        m += 1
                # V = G W2 (G symmetric -> its bf16 copy is its own lhsT)
                nc.tensor.wait_ge(s_w, 16)
                nc.tensor.wait_ge(s_dve, 2)  # gs merged
                nc.tensor.matmul(
                    v_ps[:, :128], gs[:], W2, start=True, stop=True
                ).then_inc(s_pe, 1)  # 33
                # P2 = U V
                nc.tensor.wait_ge(s_dve, 3)  # v copied
                nc.tensor.matmul(
                    p2_ps[:, :128], UT, v_sb[:], start=True, stop=True
                ).then_inc(s_pe, 1)  # 34
                # out = X P2: lhsT slices straight from host-transposed X
                nc.tensor.wait_ge(s_dve, 4)  # p2 copied
                nc.tensor.wait_ge(s_x[0], 16)
                for k in range(16):  # 35-50
                    if k == 8:
                        nc.tensor.wait_ge(s_x[1], 16)
                    nc.tensor.matmul(
                        o_ps[k // 4][:, (k % 4) * D : (k % 4 + 1) * D],
                        xt_sb[:, k * 128 : (k + 1) * 128],
                        p2_sb[:],
                        start=True,
                        stop=True,
                    ).then_inc(s_pe, 1)

            @block.vector
            def _(ve):
                # TensorTensor may read only one PSUM operand: stage bank A
                # in SBUF (overlaps the final G matmul), then add bank B.
                nc.vector.wait_ge(s_pe, 31)  # ga's accumulation done
                nc.vector.tensor_copy(ga_sb[:], ga_ps[:, :128]).then_inc(
                    s_dve, 1
                )
                nc.vector.wait_ge(s_pe, 32)
                nc.vector.tensor_add(
                    gs[:], ga_sb[:], gb_ps[:, :128]
                ).then_inc(s_dve, 1)
                nc.vector.wait_ge(s_pe, 33)
                nc.vector.tensor_copy(v_sb[:], v_ps[:, :128]).then_inc(
                    s_dve, 1
                )
                nc.vector.wait_ge(s_pe, 34)
                nc.vector.tensor_copy(p2_sb[:], p2_ps[:, :128]).then_inc(
                    s_dve, 1
                )
                nc.vector.wait_ge(s_pe, 38)  # out group 0 done (b4)
                nc.vector.tensor_copy(
                    o_sb[0][:].rearrange("p n d -> p (n d)"), o_ps[0][:]
                ).then_inc(s_dve, 1)
                nc.vector.wait_ge(s_pe, 46)  # out group 2 done (b6)
                nc.vector.tensor_copy(
                    o_sb[2][:].rearrange("p n d -> p (n d)"), o_ps[2][:]
                ).then_inc(s_dve, 1)

    nc.compile()
    return nc


def _get_nc():
    if "nc" not in _CACHE:
        _CACHE["nc"] = build_raw()
    return _CACHE["nc"]


def _prep_in_maps(inputs: dict):
    bf16 = ml_dtypes.bfloat16
    context = np.ascontiguousarray(inputs["context"]).astype(bf16)
    X = np.ascontiguousarray(inputs["X"]).astype(np.float32)
    Wq = np.ascontiguousarray(inputs["Wq"]).astype(np.float32)
    Wk = np.ascontiguousarray(inputs["Wk"]).astype(np.float32)
    Wv = np.ascontiguousarray(inputs["Wv"]).astype(np.float32)

    UT = Wk.T @ Wq  # (Wq^T Wk)^T
    W2 = SCALE * Wv.T
    w_host = np.ascontiguousarray(np.concatenate([UT, W2], axis=1).astype(bf16))

    Xb = X.astype(bf16)
    in_maps = []
    for c in range(N_CORES):
        b, h = divmod(c, 2)
        xt = np.ascontiguousarray(Xb[b, h * SQ_SHARD : (h + 1) * SQ_SHARD, :].T)
        in_maps.append(
            {"xt": xt, "ctx": np.ascontiguousarray(context[b]), "w": w_host}
        )
    return in_maps


def _unpermute(dev: np.ndarray) -> np.ndarray:
    # dev row g*512 + p*4 + j holds true row g*512 + j*128 + p
    return dev.reshape(4, 128, 4, D).transpose(0, 2, 1, 3).reshape(SQ_SHARD, D)


def _run(inputs: dict, trace: bool = False, **kw):
    in_maps = _prep_in_maps(inputs)
    nc = _get_nc()
    res = run_bass_kernel_spmd(
        nc, in_maps, core_ids=list(range(N_CORES)), trace=trace, **kw
    )
    out = np.empty((B, SQ, D), dtype=np.float32)
    for c in range(N_CORES):
        b, h = divmod(c, 2)
        out[b, h * SQ_SHARD : (h + 1) * SQ_SHARD, :] = _unpermute(
            res.results[c]["out"]
        ).astype(np.float32)
    return out, res


def kernel(**inputs: np.ndarray) -> np.ndarray:
    if os.environ.get("BASS_TRACE"):
        _install_axon_ntff_shim()
    try:
        out, _ = _run(inputs, trace=False)
    except Exception:
        # transient NRT device errors have been observed once across many
        # runs; one retry on a fresh execution
        out, _ = _run(inputs, trace=False)
    return out


if __name__ == "__main__":
    rng = np.random.default_rng(0)
    ins = {
        "context": rng.standard_normal((B, SKV, D)).astype(np.float32),
        "X": rng.standard_normal((B, SQ, D)).astype(np.float32),
        "Wq": (rng.standard_normal((D, D)) / np.sqrt(D)).astype(np.float32),
        "Wk": (rng.standard_normal((D, D)) / np.sqrt(D)).astype(np.float32),
        "Wv": (rng.standard_normal((D, D)) / np.sqrt(D)).astype(np.float32),
    }
    got = kernel(**ins)
    q = ins["X"] @ ins["Wq"].T
    k = ins["context"] @ ins["Wk"].T
    v = ins["context"] @ ins["Wv"].T
    w = np.einsum("bse,bte->bst", q, k) * SCALE
    want = np.einsum("bst,bte->bse", w, v)
    rel = np.linalg.norm(got - want) / np.linalg.norm(want)
    print("rel err vs numpy:", rel)


# revision 11
# speedup vs baseline: 1.1948x; 1.0619x over previous
"""Cross-attention without softmax on 8 trn2 NeuronCores.

Reference computes out = (X Wq^T) (C Wk^T)^T (C Wv^T) * D^-0.5 per batch.
With no softmax the product reassociates:

    out_b = X_b @ P2_b,  P2_b = U G_b W2,  U = Wq^T Wk,
    G_b = C_b^T C_b,     W2 = D^-0.5 Wv^T

U and W2 are weight-only and precomputed on the host. The device computes
G (32 accumulating 128x128 matmuls alternating between two PSUM banks so
they pipeline, fed by the ctx DMA stream), then the tiny chain
V = G W2 (G is symmetric so its SBUF copy is its own lhsT) and
P2 = U V, and finally out = X P2 as 16 matmuls whose lhsT slices come
straight from a host-pre-transposed X — no on-device transposes, no
Q' intermediate, minimal PSUM round-trips.

Sharding: batch (4) x query-half (2) -> 8 cores; each core redundantly
computes its batch's G (no collectives). I/O is pre-cast to bf16 on the
host; accumulation stays fp32 in PSUM.

Layouts: ctx row-tiles use the permuted grouping (partition p holds rows
base + p*r + j) so ctx DMA runs >=512B-contiguous per partition; G's
row-sum is invariant to that permutation. Chunks shrink toward the end
(1024,1024,768,768,256,256 rows) so G finishes almost immediately after
the last ctx byte. Out rows are stored in a device-friendly permuted
order (dev row g*512+p*4+j holds true row g*512+j*128+p,
1KB-contiguous stores) and un-permuted on the host.
"""

import os
import sys
import types

import numpy as np

_TRN_REPO = "/opt/trn_rl_repo"
if _TRN_REPO not in sys.path and not any("trn_rl_repo" in p for p in sys.path):
    sys.path.insert(0, _TRN_REPO)

import ml_dtypes  # noqa: E402

import concourse.bass as bass  # noqa: E402
import concourse.mybir as mybir  # noqa: E402
from concourse import bacc  # noqa: E402
from concourse.bass_utils import run_bass_kernel_spmd  # noqa: E402

B, SQ, SKV, D = 4, 4096, 4096, 128
N_CORES = 8
SQ_SHARD = SQ // (N_CORES // B)  # 2048
SCALE = float(D) ** -0.5
F32 = mybir.dt.float32
BF16 = mybir.dt.bfloat16

_CACHE: dict = {}


def _install_axon_ntff_shim():
    try:
        import antenv.axon_hooks  # noqa: F401

        return
    except Exception:
        pass
    try:
        from trn_agent_boot.trn_boot import _ntff_profile_via_ctypes

        import antenv

        hook = _ntff_profile_via_ctypes("/opt/axon/libaxon_pjrt.so")
        mod = types.ModuleType("antenv.axon_hooks")
        mod._hook = hook
        mod.get_axon_ntff_profile_hook = lambda: mod._hook

        def _set(h):
            mod._hook = h

        mod.set_axon_ntff_profile_hook = _set
        antenv.axon_hooks = mod
        sys.modules["antenv.axon_hooks"] = mod
    except Exception:
        pass

    try:
        import concourse.bass_utils as bu

        bu.upload_artifacts = lambda tmpdir: f"file://{tmpdir}"
    except Exception:
        pass


# ctx chunk row counts; 128*r rows each, sum = 4096
CTX_R = (8, 8, 6, 6, 2, 2)
NCC = len(CTX_R)

# PE clock-ramp tuning (see fill() in build_raw)
N_WARM = int(os.environ.get("KERNEL_WARMUP", "26"))
N_CHAIN = int(os.environ.get("KERNEL_CHAINFILL", "3"))


def build_raw():
    """Hand-scheduled raw-bass kernel. Per-core inputs:
    xt (128, 2048) = X-shard transposed, ctx (4096, 128),
    w (128, 256) = [U^T | W2]; output out (2048, 128), permuted rows.

    DMA: sync HW queue: ctx chunks 0,2,4 + xt half 0, stores o0,o2;
    scalar HW queue: ctx chunks 1,3,5 + xt half 1, stores o1,o3 (plus
    o1/o3 PSUM copies); gpsimd SW queue: w only.

    Cumulative semaphore schedules (value after the op):
      PE (s_pe): G 1-32 (chunk ends at 8,16,22,28,30,32), V 33, P2 34,
                 out 35-50 (4 groups of 4, banks b4-b7)
      DVE (s_dve): gs merge 1, v 2, p2 3, o0 copy 4, o2 copy 5

    PSUM banks: b0/b1 = G even/odd accumulators; b2 = V; b3 = P2;
    b4-b7 = out groups 0-3. Same-bank PE-write vs engine-read pairs are
    serialized by the s_pe waits listed above.
    """
    from contextlib import ExitStack

    cdt = BF16

    nc = bacc.Bacc(None, target_bir_lowering=False, debug=False)
    xt_ext = nc.declare_dram_parameter("xt", [D, SQ_SHARD], cdt, isOutput=False)
    c_ext = nc.declare_dram_parameter("ctx", [SKV, D], cdt, isOutput=False)
    w_ext = nc.declare_dram_parameter("w", [D, 2 * D], cdt, isOutput=False)
    out_ext = nc.declare_dram_parameter("out", [SQ_SHARD, D], cdt, isOutput=True)

    NOG = 4  # out store groups
    OT = SQ_SHARD // 128 // NOG  # 4 tiles per group

    ctx_view = []
    row = 0
    for r in CTX_R:
        ctx_view.append(
            c_ext[row : row + 128 * r, :].rearrange("(p r) d -> p r d", p=128)
        )
        row += 128 * r
    out_view = [
        out_ext[g * 512 : (g + 1) * 512, :].rearrange("(p r) d -> p r d", p=128)
        for g in range(NOG)
    ]

    es = ExitStack()
    _n = [0]

    def sb(shape, dt, name=None):
        _n[0] += 1
        return es.enter_context(nc.sbuf_tensor(name or f"sb{_n[0]}", shape, dt))

    def pst(shape, dt, name=None):
        _n[0] += 1
        return es.enter_context(nc.psum_tensor(name or f"ps{_n[0]}", shape, dt))

    def sem(name):
        return es.enter_context(nc.semaphore(name))

    with es:
        w_sb = sb([D, 2 * D], cdt, "w_sb")
        cc = [sb([128, r, D], cdt, f"cc{i}") for i, r in enumerate(CTX_R)]
        xt_sb = sb([D, SQ_SHARD], cdt, "xt_sb")
        warm_sb = sb([D, D], cdt, "warm_sb")
        ga_sb = sb([D, D], F32, "ga_sb")
        gs = sb([D, D], cdt, "gs")
        v_sb = sb([D, D], cdt, "v_sb")
        p2_sb = sb([D, D], cdt, "p2_sb")
        o_sb = [sb([128, OT, D], cdt, f"o_sb{i}") for i in range(NOG)]

        ga_ps = pst([128, 512], F32)  # b0
        gb_ps = pst([128, 512], F32)  # b1
        v_ps = pst([128, 512], F32)  # b2
        p2_ps = pst([128, 512], F32)  # b3
        o_ps = [pst([128, 512], F32) for _ in range(NOG)]  # b4-b7

        s_pe = sem("s_pe")
        s_dve = sem("s_dve")
        s_w = sem("s_w")
        s_c = [sem(f"s_c{i}") for i in range(NCC)]
        s_x = [sem(f"s_x{i}") for i in range(2)]
        s_st = sem("s_st")

        UT = w_sb[:, :D]
        W2 = w_sb[:, D:]

        with nc.Block() as block:

            @block.sync
            def _(sync):
                for i in (0, 2, 4):
                    nc.sync.dma_start(cc[i][:], ctx_view[i]).then_inc(s_c[i], 16)
                nc.sync.dma_start(
                    xt_sb[:, : SQ_SHARD // 2], xt_ext[:, : SQ_SHARD // 2]
                ).then_inc(s_x[0], 16)
                nc.sync.wait_ge(s_dve, 5)  # o0 copied
                nc.sync.dma_start(out_view[0], o_sb[0][:]).then_inc(s_st, 16)
                nc.sync.wait_ge(s_dve, 6)  # o2 copied
                nc.sync.dma_start(out_view[2], o_sb[2][:]).then_inc(s_st, 16)
                nc.sync.wait_ge(s_st, 64)

            @block.scalar
            def _(sc):
                for i in (1, 3, 5):
                    nc.scalar.dma_start(cc[i][:], ctx_view[i]).then_inc(
                        s_c[i], 16
                    )
                nc.scalar.dma_start(
                    xt_sb[:, SQ_SHARD // 2 :], xt_ext[:, SQ_SHARD // 2 :]
                ).then_inc(s_x[1], 16)
                nc.scalar.wait_ge(s_pe, 42)  # out group 1 done (b5)
                nc.scalar.copy(
                    o_sb[1][:].rearrange("p n d -> p (n d)"), o_ps[1][:]
                )
                nc.scalar.dma_start(out_view[1], o_sb[1][:]).then_inc(s_st, 16)
                nc.scalar.wait_ge(s_pe, 50)  # out group 3 done (b7)
                nc.scalar.copy(
                    o_sb[3][:].rearrange("p n d -> p (n d)"), o_ps[3][:]
                )
                nc.scalar.dma_start(out_view[3], o_sb[3][:]).then_inc(s_st, 16)
                nc.scalar.wait_ge(s_st, 64)

            @block.gpsimd
            def _(gp):
                nc.gpsimd.dma_start(w_sb[:], w_ext[:]).then_inc(s_w, 16)

            @block.tensor
            def _(te):
                def fill(n):
                    # PE DVFS: full clock only after ~3us of continuous
                    # execution, and stalls drop it back. Dummy matmuls on
                    # scratch data ramp the clock during DMA waits and hold
                    # it through the V/P2 chain gaps. Results are discarded
                    # (b7 is overwritten by out group 3 later).
                    for _ in range(n):
                        nc.tensor.matmul(
                            o_ps[3][:, :128],
                            warm_sb[:],
                            warm_sb[:],
                            start=True,
                            stop=True,
                        )

                fill(N_WARM)
                # G: alternate PSUM banks per matmul so consecutive matmuls
                # pipeline instead of serializing on one bank's accumulator
                m = 0
                n_mm = sum(CTX_R)
                for i, r in enumerate(CTX_R):
                    nc.tensor.wait_ge(s_c[i], 16)
                    for j in range(r):
                        bank = ga_ps if m % 2 == 0 else gb_ps
                        nc.tensor.matmul(
                            bank[:, :128],
                            cc[i][:, j, :],
                            cc[i][:, j, :],
                            start=(m < 2),
                            stop=(m >= n_mm - 2),
                        ).then_inc(s_pe, 1)
                        m += 1
                # V = G W2 (G symmetric -> its bf16 copy is its own lhsT)
                nc.tensor.wait_ge(s_w, 16)
                fill(N_CHAIN)  # hold clock while vector merges G
                nc.tensor.wait_ge(s_dve, 2)  # gs merged
                nc.tensor.matmul(
                    v_ps[:, :128], gs[:], W2, start=True, stop=True
                ).then_inc(s_pe, 1)  # 33
                fill(N_CHAIN)  # hold clock while vector copies V
                # P2 = U V
                nc.tensor.wait_ge(s_dve, 3)  # v copied
                nc.tensor.matmul(
                    p2_ps[:, :128], UT, v_sb[:], start=True, stop=True
                ).then_inc(s_pe, 1)  # 34
                fill(N_CHAIN)  # hold clock while vector copies P2
                # out = X P2: lhsT slices straight from host-transposed X
                nc.tensor.wait_ge(s_dve, 4)  # p2 copied
                nc.tensor.wait_ge(s_x[0], 16)
                for k in range(16):  # 35-50
                    if k == 8:
                        nc.tensor.wait_ge(s_x[1], 16)
                    nc.tensor.matmul(
                        o_ps[k // 4][:, (k % 4) * D : (k % 4 + 1) * D],
                        xt_sb[:, k * 128 : (k + 1) * 128],
                        p2_sb[:],
                        start=True,
                        stop=True,
                    ).then_inc(s_pe, 1)

            @block.vector
            def _(ve):
                # TensorTensor may read only one PSUM operand: stage bank A
                # in SBUF (overlaps the final G matmul), then add bank B.
                nc.vector.wait_ge(s_pe, 31)  # ga's accumulation done
                nc.vector.tensor_copy(ga_sb[:], ga_ps[:, :128]).then_inc(
                    s_dve, 1
                )
                nc.vector.wait_ge(s_pe, 32)
                nc.vector.tensor_add(
                    gs[:], ga_sb[:], gb_ps[:, :128]
                ).then_inc(s_dve, 1)
                nc.vector.wait_ge(s_pe, 33)
                nc.vector.tensor_copy(v_sb[:], v_ps[:, :128]).then_inc(
                    s_dve, 1
                )
                nc.vector.wait_ge(s_pe, 34)
                nc.vector.tensor_copy(p2_sb[:], p2_ps[:, :128]).then_inc(
                    s_dve, 1
                )
                nc.vector.wait_ge(s_pe, 38)  # out group 0 done (b4)
                nc.vector.tensor_copy(
                    o_sb[0][:].rearrange("p n d -> p (n d)"), o_ps[0][:]
                ).then_inc(s_dve, 1)
                nc.vector.wait_ge(s_pe, 46)  # out group 2 done (b6)
                nc.vector.tensor_copy(
                    o_sb[2][:].rearrange("p n d -> p (n d)"), o_ps[2][:]
                ).then_inc(s_dve, 1)

    nc.compile()
    return nc


def _get_nc():
    if "nc" not in _CACHE:
        _CACHE["nc"] = build_raw()
    return _CACHE["nc"]


def _prep_in_maps(inputs: dict):
    bf16 = ml_dtypes.bfloat16
    context = np.ascontiguousarray(inputs["context"]).astype(bf16)
    X = np.ascontiguousarray(inputs["X"]).astype(np.float32)
    Wq = np.ascontiguousarray(inputs["Wq"]).astype(np.float32)
    Wk = np.ascontiguousarray(inputs["Wk"]).astype(np.float32)
    Wv = np.ascontiguousarray(inputs["Wv"]).astype(np.float32)

    UT = Wk.T @ Wq  # (Wq^T Wk)^T
    W2 = SCALE * Wv.T
    w_host = np.ascontiguousarray(np.concatenate([UT, W2], axis=1).astype(bf16))

    Xb = X.astype(bf16)
    in_maps = []
    for c in range(N_CORES):
        b, h = divmod(c, 2)
        xt = np.ascontiguousarray(Xb[b, h * SQ_SHARD : (h + 1) * SQ_SHARD, :].T)
        in_maps.append(
            {"xt": xt, "ctx": np.ascontiguousarray(context[b]), "w": w_host}
        )
    return in_maps


def _unpermute(dev: np.ndarray) -> np.ndarray:
    # dev row g*512 + p*4 + j holds true row g*512 + j*128 + p
    return dev.reshape(4, 128, 4, D).transpose(0, 2, 1, 3).reshape(SQ_SHARD, D)


def _run(inputs: dict, trace: bool = False, **kw):
    in_maps = _prep_in_maps(inputs)
    nc = _get_nc()
    res = run_bass_kernel_spmd(
        nc, in_maps, core_ids=list(range(N_CORES)), trace=trace, **kw
    )
    out = np.empty((B, SQ, D), dtype=np.float32)
    for c in range(N_CORES):
        b, h = divmod(c, 2)
        out[b, h * SQ_SHARD : (h + 1) * SQ_SHARD, :] = _unpermute(
            res.results[c]["out"]
        ).astype(np.float32)
    return out, res


def kernel(**inputs: np.ndarray) -> np.ndarray:
    if os.environ.get("BASS_TRACE"):
        _install_axon_ntff_shim()
    try:
        out, _ = _run(inputs, trace=False)
    except Exception:
        # transient NRT device errors have been observed once across many
        # runs; one retry on a fresh execution
        out, _ = _run(inputs, trace=False)
    return out


if __name__ == "__main__":
    rng = np.random.default_rng(0)
    ins = {
        "context": rng.standard_normal((B, SKV, D)).astype(np.float32),
        "X": rng.standard_normal((B, SQ, D)).astype(np.float32),
        "Wq": (rng.standard_normal((D, D)) / np.sqrt(D)).astype(np.float32),
        "Wk": (rng.standard_normal((D, D)) / np.sqrt(D)).astype(np.float32),
        "Wv": (rng.standard_normal((D, D)) / np.sqrt(D)).astype(np.float32),
    }
    got = kernel(**ins)
    q = ins["X"] @ ins["Wq"].T
    k = ins["context"] @ ins["Wk"].T
    v = ins["context"] @ ins["Wv"].T
    w = np.einsum("bse,bte->bst", q, k) * SCALE
    want = np.einsum("bst,bte->bse", w, v)
    rel = np.linalg.norm(got - want) / np.linalg.norm(want)
    print("rel err vs numpy:", rel)
